# revision 37
# baseline (speedup 1.0000x reference)
"""Trainium2 Bass kernel for nn_AdjointManifoldBlock.

Reference computes 10 RK4 steps of:
    dx/dt = v ; dv/dt = -gamma,  gamma = ((v@Wa)*(v@Wb)*tanh(x@Wx)) @ Wc

Key restructuring: the dynamics are linear in (x, v) except the rank-space
(64-dim) elementwise product. Tracking per-token rank-space state
    a = v@Wa, b = v@Wb, h = x@Wx, w = (dt/2) * (v@Wx)
every RK4 stage update is a [64,64] GEMM with composite matrices
    Caa = Wc@Wa, Cab = Wc@Wb, Cax = Wc@Wx
and the DIM-space state is only touched at the very end:
    x_T = x0 + T*v0 - (dt^2/6) * Q @ Wc        (T = STEPS*dt = 1.0)
    v_T = v0 -        (dt/6)  * S @ Wc
with S = sum_n S_n, Q = sum_n [(STEPS-1-n) S_n + P_n],
S_n = c1+2c2+2c3+c4, P_n = c1+c2+c3 (RK4 stage coefficients of step n).

Per-step recurrences (c_s = a_s*b_s*tanh(h_s)):
    a2 = a1 - (dt/2) c1@Caa ; a3 = a1 - (dt/2) c2@Caa ; a4 = a1 - dt c3@Caa
    a1' = a1 - (dt/6) S_n@Caa                       (same shape for b)
    h2 = h1 + w ; h3 = h1 + w - (dt^2/4) c1@Cax
    h4 = h1 + 2w - (dt^2/2) c2@Cax
    h1' = h1 + 2w - (dt^2/6) P_n@Cax ; w' = w - (dt^2/12) S_n@Cax

Mapping to the NeuronCore (per core: 1024 tokens, data-parallel over 8):
- two 512-token tiles are partition-stacked: rank-space tensors are
  [128, 512] (tile A ranks on partitions 0:64, tile B on 64:128)
- rank GEMMs are single full-array fp32r matmuls with block-diagonal
  [[sC,0],[0,sC]] weights (K=128 covers both tiles at once)
- a and h live in PSUM banks updated purely by PE accumulation; "+w" uses
  a block-diagonal identity matmul; S and Q accumulate in PSUM via
  scaled-identity matmuls; b-deltas and w-deltas land in scratch banks
- per stage: m = b*tanh(h) runs early on GPSIMD; c = a*m on DVE is the
  only op on the serial stage chain
- the step loop is column-split into NSPLIT independent chains (shared
  PSUM banks, disjoint column ranges) to hide cross-engine latency
- entry: PE-transposes x,v into feature-major chunks feeding DIM->RANK
  GEMMs; exit: rank-space accumulators (as stationary operands, sliced
  by token block) x Wc produce token-major output directly; x0+v0 is
  pre-added in place on GPSIMD during the steps.
"""

import json
import numpy as np

DIM = 1024
RANK = 64
STEPS = 10
DT = 0.1
BATCH, SEQ = 4, 2048
NCORES = 8
TPC = (BATCH * SEQ) // NCORES  # tokens per core = 1024
N = TPC // 2  # tokens per stacked half = 512
NCH = DIM // 128  # feature chunks = 8
NSPLIT = 2  # independent step-loop chains (column split); fp32r needs N/NSPLIT>=256

D2 = DT * DT

CAA_SCALES = [-DT / 2, DT / 2, -DT, -DT / 6, -DT / 3, 2 * DT / 3]
CAB_SCALES = [-DT / 2, -DT, -DT / 6, -DT / 3]
CAX_SCALES = [-D2 / 4, D2 / 4, -D2 / 2, D2 / 3, -D2 / 6, -D2 / 12]
IBD_SCALES = sorted(
    {1.0, 2.0}
    | {float(10 - n) for n in range(STEPS)}
    | {float(19 - 2 * n) for n in range(STEPS)}
    | {float(9 - n) for n in range(STEPS) if 9 - n > 0}
)
NV = len(CAA_SCALES) + len(CAB_SCALES) + len(CAX_SCALES) + len(IBD_SCALES)


def _vidx(kind, scale):
    if kind == "caa":
        return CAA_SCALES.index(scale)
    if kind == "cab":
        return len(CAA_SCALES) + CAB_SCALES.index(scale)
    if kind == "cax":
        return len(CAA_SCALES) + len(CAB_SCALES) + CAX_SCALES.index(scale)
    if kind == "ibd":
        return (
            len(CAA_SCALES)
            + len(CAB_SCALES)
            + len(CAX_SCALES)
            + IBD_SCALES.index(float(scale))
        )
    raise KeyError(kind)


# ---------------------------------------------------------------- host consts


def _host_consts(Wa, Wb, Wx, Wc):
    Wa64 = np.asarray(Wa, np.float64)
    Wb64 = np.asarray(Wb, np.float64)
    Wx64 = np.asarray(Wx, np.float64)
    Wc64 = np.asarray(Wc, np.float64)

    Caa = Wc64 @ Wa64  # [64, 64]; row index = coeff rank (contraction side)
    Cab = Wc64 @ Wb64
    Cax = Wc64 @ Wx64
    I64 = np.eye(RANK)

    cmp_mats = (
        [Caa * sc for sc in CAA_SCALES]
        + [Cab * sc for sc in CAB_SCALES]
        + [Cax * sc for sc in CAX_SCALES]
        + [I64 * sc for sc in IBD_SCALES]
    )
    bdarr = np.stack(cmp_mats).astype(np.float32)  # [NV, 64, 64]
    bdarr = np.ascontiguousarray(bdarr.transpose(1, 0, 2))  # [64, NV, 64]

    # start weights: tensor t in (Wa, Wb, Wx, (dt/2)Wx), chunk k in 0..7
    stk = np.stack(
        [W.reshape(NCH, 128, RANK) for W in (Wa64, Wb64, Wx64, (DT / 2) * Wx64)]
    )  # [4, 8, 128, 64]
    wsa = np.ascontiguousarray(
        stk.transpose(2, 0, 1, 3).reshape(128, 4 * NCH, RANK)
    ).astype(np.float32)  # [128, 32, 64] (A-tile weights, natural)
    # B-tile weights are the same data placed in columns 64:128 of a
    # zeroed [128, 32, 128]; ship only the data half, zero-fill on device.
    wcv = np.asarray(-(DT / 6) * Wc64, np.float32)  # [64, 1024]
    wcx = np.asarray(-(D2 / 6) * Wc64, np.float32)
    ident = np.eye(128, dtype=np.float32)

    return {"bd": bdarr, "wsa": wsa, "wcv": wcv, "wcx": wcx, "ident": ident}


# ----------------------------------------------------------- BIR wait postpass


def _split_waits(data: bytes) -> bytes:
    """This walrus build accepts only one inline sync wait per instruction;
    move excess waits onto NoOps inserted before the instruction (the
    engine sequencer processes them in order, so semantics are identical)."""
    bir = json.loads(data)
    for fn in bir["functions"]:
        for blk in fn["blocks"]:
            out = []
            k = 0
            for inst in blk["instructions"]:
                si = inst.get("sync_info")
                if si and len(si.get("on_wait", [])) > 1:
                    waits = si["on_wait"]
                    pre = []
                    while len(waits) > 1:
                        chunk, waits = waits[:1], waits[1:]
                        k += 1
                        pre.append(
                            {
                                "name": f"{inst['name']}-w{k}",
                                "opcode": "NoOp",
                                "engine": inst["engine"],
                                "ins": [],
                                "outs": [],
                                "sync_info": {"on_wait": chunk, "on_update": []},
                            }
                        )
                    si["on_wait"] = waits
                    out.extend(pre)
                out.append(inst)
            blk["instructions"] = out
    return json.dumps(bir).encode()


# ---------------------------------------------------------------- bass builder

_NC_CACHE = None
DEBUG = False
DEBUG_STEP = 0


def _build_bass():
    global _NC_CACHE
    if _NC_CACHE is not None:
        return _NC_CACHE

    import concourse.bass as bass
    import concourse.tile as tile
    import concourse.mybir as mybir

    F32 = mybir.dt.float32
    F32R = mybir.dt.float32r
    TANH = mybir.ActivationFunctionType.Tanh
    COPY = mybir.ActivationFunctionType.Copy

    nc = bass.Bass("TRN2", target_bir_lowering=False, debug=False, num_devices=1)

    xin = nc.dram_tensor("xin", [TPC, DIM], F32, kind="ExternalInput").ap()
    vin = nc.dram_tensor("vin", [TPC, DIM], F32, kind="ExternalInput").ap()
    bdm = nc.dram_tensor("bd", [64, NV, 64], F32, kind="ExternalInput").ap()
    wsa = nc.dram_tensor("wsa", [128, 4 * NCH, RANK], F32, kind="ExternalInput").ap()
    wcv = nc.dram_tensor("wcv", [RANK, DIM], F32, kind="ExternalInput").ap()
    wcx = nc.dram_tensor("wcx", [RANK, DIM], F32, kind="ExternalInput").ap()
    idn = nc.dram_tensor("ident", [128, 128], F32, kind="ExternalInput").ap()
    xout = nc.dram_tensor("xout", [TPC, DIM], F32, kind="ExternalOutput").ap()
    vout = nc.dram_tensor("vout", [TPC, DIM], F32, kind="ExternalOutput").ap()
    dbg = {}
    if DEBUG:
        for nm in ("a1", "h1", "b1", "w", "wd", "c1", "c2", "c3", "c4", "S", "Q"):
            dbg[nm] = nc.dram_tensor(
                f"dbg_{nm}", [128, N], F32, kind="ExternalOutput"
            ).ap()

    NC2 = N // NSPLIT  # columns per chain

    with tile.TileContext(nc) as tc:
        with (
            tc.tile_pool(name="consts", bufs=1) as consts,
            tc.tile_pool(name="work", bufs=1) as work,
            tc.tile_pool(name="bpool", bufs=3) as bpool,
            tc.tile_pool(name="cpool", bufs=6) as cpool,
            tc.tile_pool(name="tpool", bufs=4) as tpool,
            tc.tile_pool(name="mpool", bufs=4) as mpool,
            tc.tile_pool(name="ps_main", bufs=1, space="PSUM") as ps_main,
        ):
            # ---------------- input loads (v first: its transposes and
            # GEMMs are the front of the pipeline)
            s_xtok = consts.tile([128, NCH, DIM], F32, tag="xtok")
            s_vtok = consts.tile([128, NCH, DIM], F32, tag="vtok")

            # persistent PSUM accumulators; memset clears both values and
            # makes any stale has_written state harmless, so every matmul
            # below can use start=False in any order.
            # chain-private a and h banks: avoids the PSUM same-bank
            # PE-write / engine-read serialization between the two chains.
            # cols 0:NC2 of Ba hold a; cols NC2:2NC2 hold the cumulative
            # w-delta (w_n = w0 + wdcum), which needs no per-step clearing.
            p_a = ps_main.tile([128, N], F32, tag="Ba")
            p_a2 = ps_main.tile([128, N], F32, tag="Ba2")
            p_h = ps_main.tile([128, NC2], F32, tag="Bh")
            p_h2 = ps_main.tile([128, NC2], F32, tag="Bh2")
            p_b0 = ps_main.tile([128, N], F32, tag="BS")
            p_w0 = ps_main.tile([128, N], F32, tag="BQ")
            for p in (p_a, p_a2, p_h, p_h2, p_b0, p_w0):
                nc.vector.memset(p[:], 0.0)
            p_a_ch = [p_a, p_a2]
            p_h_ch = [p_h, p_h2]

            def bdw(kind, scale):
                return s_bd[:, _vidx(kind, scale), :].bitcast(F32R)

            # ---------------- entry: transposes + DIM->RANK GEMMs
            with (
                tc.tile_pool(name="entry", bufs=1) as entry,
                tc.tile_pool(name="stream", bufs=2) as stream,
                tc.tile_pool(name="ps_tr", bufs=2, space="PSUM") as ps_tr,
            ):
                s_id = entry.tile([128, 128], F32, tag="ident")
                nc.sync.dma_start(s_id[:].bitcast(F32R), idn[:].bitcast(F32R))
                s_wsa = entry.tile([128, 4 * NCH, RANK], F32, tag="wsa")
                nc.sync.dma_start(s_wsa[:].bitcast(F32R), wsa[:].bitcast(F32R))
                # A-half token blocks (0-3) of both tensors first: the
                # A-half transpose+GEMM pipeline starts while B still loads
                s_wsb = entry.tile([128, 4 * NCH, 128], F32, tag="wsb")
                nc.vector.memset(s_wsb[:], 0.0)
                for tb in range(NCH // 2):
                    nc.sync.dma_start(
                        s_vtok[:, tb, :].bitcast(F32R),
                        vin[tb * 128 : (tb + 1) * 128, :].bitcast(F32R),
                    )
                for tb in range(NCH // 2):
                    nc.sync.dma_start(
                        s_xtok[:, tb, :].bitcast(F32R),
                        xin[tb * 128 : (tb + 1) * 128, :].bitcast(F32R),
                    )
                # B-tile padded weights: zero-fill + one strided DMA of the
                # data half into columns 64:128
                nc.sync.dma_start(
                    s_wsb[:, :, 64:128].bitcast(F32R), wsa[:].bitcast(F32R)
                )
                for tb in range(NCH // 2, NCH):
                    nc.sync.dma_start(
                        s_vtok[:, tb, :].bitcast(F32R),
                        vin[tb * 128 : (tb + 1) * 128, :].bitcast(F32R),
                    )
                for tb in range(NCH // 2, NCH):
                    nc.sync.dma_start(
                        s_xtok[:, tb, :].bitcast(F32R),
                        xin[tb * 128 : (tb + 1) * 128, :].bitcast(F32R),
                    )

                # late consts (not needed until steps / exit); block-diag
                # [NV][128,128] built from compact [NV][64,64]: zero-fill,
                # then two strided DMAs into the diagonal blocks
                s_bd = consts.tile([128, NV, 128], F32, tag="bd")
                nc.vector.memset(s_bd[:], 0.0)
                nc.sync.dma_start(
                    s_bd[0:64, :, 0:64].bitcast(F32R), bdm[:].bitcast(F32R)
                )
                nc.sync.dma_start(
                    s_bd[64:128, :, 64:128].bitcast(F32R), bdm[:].bitcast(F32R)
                )
                s_wcv = consts.tile([128, DIM], F32, tag="wcv")
                nc.sync.dma_start(
                    s_wcv[0:64, :].bitcast(F32R), wcv[:].bitcast(F32R)
                )
                nc.sync.dma_start(
                    s_wcv[64:128, :].bitcast(F32R), wcv[:].bitcast(F32R)
                )
                s_wcx = consts.tile([128, DIM], F32, tag="wcx")
                nc.sync.dma_start(
                    s_wcx[0:64, :].bitcast(F32R), wcx[:].bitcast(F32R)
                )
                nc.sync.dma_start(
                    s_wcx[64:128, :].bitcast(F32R), wcx[:].bitcast(F32R)
                )

                for half in range(2):
                    for k in range(NCH):
                        vT = stream.tile([128, N], F32, tag="vT")
                        xT = stream.tile([128, N], F32, tag="xT")
                        for src_tok, dst in ((s_vtok, vT), (s_xtok, xT)):
                            p_tr = ps_tr.tile([128, N], F32R, tag="tr")
                            for q in range(4):
                                tb = half * 4 + q
                                nc.tensor.transpose(
                                    p_tr[:, q * 128 : (q + 1) * 128],
                                    src_tok[
                                        :, tb, k * 128 : (k + 1) * 128
                                    ].bitcast(F32R),
                                    s_id[:].bitcast(F32R),
                                )
                            nc.scalar.activation(
                                dst[:].bitcast(F32R),
                                p_tr[:].bitcast(F32),
                                COPY,
                            )
                        # MMs consuming this half only (A-half: natural
                        # weights, M=64; B-half: zero-padded, M=128)
                        for bank, tsel, src in (
                            (p_a, 0, vT),
                            (p_b0, 1, vT),
                            (p_h, 2, xT),
                            (p_w0, 3, vT),
                        ):
                            banks = (
                                (p_a_ch if tsel == 0 else p_h_ch)
                                if tsel in (0, 2)
                                else [bank]
                            )
                            for ci, bk in enumerate(banks):
                                split = tsel in (0, 2)
                                lo = ci * NC2 if split else 0
                                cw = NC2 if split else N
                                osl = slice(0, cw)
                                if half == 0:
                                    nc.tensor.matmul(
                                        bk[0:64, osl],
                                        s_wsa[:, tsel * NCH + k, :].bitcast(
                                            F32R
                                        ),
                                        src[:, lo : lo + cw].bitcast(F32R),
                                        start=False,
                                        stop=False,
                                        skip_group_check=True,
                                    )
                                else:
                                    nc.tensor.matmul(
                                        bk[:, osl],
                                        s_wsb[:, tsel * NCH + k, :].bitcast(
                                            F32R
                                        ),
                                        src[:, lo : lo + cw].bitcast(F32R),
                                        start=False,
                                        stop=(k == NCH - 1),
                                        skip_group_check=True,
                                    )

            # b and w to SBUF (per chain); banks become the S/Q accumulators
            chains = []
            for ch in range(NSPLIT):
                sl = slice(ch * NC2, (ch + 1) * NC2)
                b1 = bpool.tile([128, NC2], F32, tag=f"b1_{ch}")
                nc.vector.tensor_copy(b1[:], p_b0[:, sl])
                w0 = bpool.tile([128, NC2], F32, tag=f"w0_{ch}")
                nc.vector.tensor_copy(w0[:].bitcast(F32R), p_w0[:, sl])
                chains.append({"b1": b1, "w0": w0, "w": w0, "sl": sl})
            p_S = ps_main.tile([128, N], F32, tag="BS")
            p_Q = ps_main.tile([128, N], F32, tag="BQ")

            # x0 += v0 happens in place on GPSIMD, spread across the steps

            # ---------------- the 10 RK4 steps, fully unrolled
            with tc.tile_pool(name="ps_step", bufs=1, space="PSUM") as ps_step:

                def mm(bank, sl, kind, scale, rhs_view, start=False, stop=False):
                    nc.tensor.matmul(
                        bank[:, sl],
                        bdw(kind, scale),
                        rhs_view,
                        start=start,
                        stop=stop,
                        skip_group_check=True,
                    )

                def step_chain(n, st, db_tiles):
                    """Emit one RK4 step for one chain; yields between stages
                    so chains can be interleaved."""
                    ch = st["ch"]
                    sl = st["sl"]
                    pa = p_a_ch[ch]
                    ph = p_h_ch[ch]
                    asl = slice(0, NC2)
                    wsl = slice(NC2, 2 * NC2)
                    last = n == STEPS - 1
                    q1, q23, q4 = float(10 - n), float(19 - 2 * n), float(9 - n)

                    def tanh(idx):
                        t = tpool.tile([128, NC2], F32, tag=f"tanh_{ch}")
                        nc.scalar.activation(t[:], ph[:, asl], TANH)
                        return t

                    def premul(b_s, t_s, stage):
                        # m = b * tanh(h): off the DVE, on GPSIMD
                        m = mpool.tile([128, NC2], F32, tag=f"m_{ch}")
                        nc.gpsimd.tensor_mul(m[:], b_s[:], t_s[:])
                        return m

                    def coeff(m_s):
                        # c = a * m: PSUM x SBUF on DVE, f32r out
                        c = cpool.tile([128, NC2], F32, tag=f"c_{ch}")
                        nc.vector.tensor_mul(c[:].bitcast(F32R), pa[:, asl], m_s[:])
                        if DEBUG and n == DEBUG_STEP:
                            st.setdefault("cdump", []).append(c)
                        return c[:].bitcast(F32R)

                    def badd(db):
                        b = bpool.tile([128, NC2], F32, tag=f"bs_{ch}")
                        nc.vector.tensor_add(b[:], st["b1"][:], db[:])
                        return b

                    # stage 1
                    t1 = tanh(1)
                    mm(ph, asl, "ibd", 1.0, st["w"][:].bitcast(F32R), stop=True)
                    t2 = tanh(2)
                    m1 = premul(st["b1"], t1, 1)
                    c1 = coeff(m1)
                    # start=True clears has_written for the WHOLE bank, so
                    # only the very first write (chain 0) may carry it
                    mm(p_S, sl, "ibd", 1.0, c1, start=(n == 0 and ch == 0))
                    mm(p_Q, sl, "ibd", q1, c1, start=(n == 0 and ch == 0))
                    mm(pa, asl, "caa", -DT / 2, c1, stop=True)  # a2
                    mm(db_tiles[0], slice(0, NC2), "cab", -DT / 2, c1, start=True, stop=True)
                    b2 = badd(db_tiles[0])
                    mm(ph, asl, "cax", -D2 / 4, c1, stop=True)  # h3
                    t3 = tanh(3)
                    if not last:
                        mm(pa, wsl, "cax", -D2 / 12, c1)
                    yield

                    # stage 2
                    m2 = premul(b2, t2, 2)
                    c2 = coeff(m2)
                    mm(p_S, sl, "ibd", 2.0, c2)
                    mm(p_Q, sl, "ibd", q23, c2)
                    mm(pa, asl, "caa", DT / 2, c1)
                    mm(pa, asl, "caa", -DT / 2, c2, stop=True)  # a3
                    mm(db_tiles[1], slice(0, NC2), "cab", -DT / 2, c2, start=True, stop=True)
                    b3 = badd(db_tiles[1])
                    mm(ph, asl, "ibd", 1.0, st["w"][:].bitcast(F32R))
                    mm(ph, asl, "cax", D2 / 4, c1)
                    mm(ph, asl, "cax", -D2 / 2, c2, stop=True)  # h4
                    t4 = tanh(4)
                    if not last:
                        mm(pa, wsl, "cax", -D2 / 6, c2)
                    yield

                    # stage 3
                    m3 = premul(b3, t3, 3)
                    c3 = coeff(m3)
                    mm(p_S, sl, "ibd", 2.0, c3)
                    mm(p_Q, sl, "ibd", q23, c3, stop=last)
                    mm(pa, asl, "caa", DT / 2, c2)
                    mm(pa, asl, "caa", -DT, c3, stop=True)  # a4
                    mm(db_tiles[2], slice(0, NC2), "cab", -DT, c3, start=True, stop=True)
                    b4 = badd(db_tiles[2])
                    if not last:
                        mm(pa, wsl, "cax", -D2 / 6, c3)
                    yield

                    # stage 4
                    m4 = premul(b4, t4, 4)
                    c4 = coeff(m4)
                    mm(p_S, sl, "ibd", 1.0, c4, stop=last)
                    if not last:
                        mm(p_Q, sl, "ibd", q4, c4)
                        mm(pa, asl, "caa", 2 * DT / 3, c3)
                        mm(pa, asl, "caa", -DT / 6, c1)
                        mm(pa, asl, "caa", -DT / 3, c2)
                        mm(pa, asl, "caa", -DT / 6, c4, stop=True)  # a1'
                        mm(ph, asl, "cax", D2 / 3, c2)
                        mm(ph, asl, "cax", -D2 / 6, c1)
                        mm(ph, asl, "cax", -D2 / 6, c3, stop=True)  # h1'
                        mm(db_tiles[3], slice(0, NC2), "cab", -DT / 6, c1, start=True)
                        mm(db_tiles[3], slice(0, NC2), "cab", -DT / 3, c2)
                        mm(db_tiles[3], slice(0, NC2), "cab", -DT / 3, c3)
                        mm(db_tiles[3], slice(0, NC2), "cab", -DT / 6, c4, stop=True)
                        nb1 = bpool.tile([128, NC2], F32, tag=f"b1_{st['ch']}")
                        nc.vector.tensor_add(nb1[:], st["b1"][:], db_tiles[3][:])
                        st["b1"] = nb1
                        mm(pa, wsl, "cax", -D2 / 12, c4, stop=True)
                        nw = bpool.tile([128, NC2], F32, tag=f"w_{st['ch']}")
                        nc.vector.tensor_add(
                            nw[:].bitcast(F32R), st["w0"][:], pa[:, wsl]
                        )
                        st["w"] = nw
                    yield

                for ch, st in enumerate(chains):
                    st["ch"] = ch
                for n in range(STEPS):
                    # shared scratch banks for this step; both chains use
                    # disjoint column halves (memset once per allocation
                    # round: values are fully written by their MMs after a
                    # region memset clears stale has_written semantics)
                    last_step = n == STEPS - 1
                    db_per_chain = []
                    for ci in range(NSPLIT):
                        da = ps_step.tile([128, NC2], F32, tag=f"db{ci}")
                        db = ps_step.tile([128, NC2], F32, tag=f"db{ci}")
                        dc = ps_step.tile([128, NC2], F32, tag=f"db{ci}")
                        dd = (
                            ps_step.tile([128, NC2], F32, tag=f"db{ci}")
                            if not last_step
                            else None
                        )
                        db_per_chain.append([da, db, dc, dd])
                    if DEBUG and n == DEBUG_STEP:
                        for ci2, st2 in enumerate(chains):
                            csl2 = slice(ci2 * NC2, (ci2 + 1) * NC2)
                            pa2_d = p_a_ch[ci2] if NSPLIT == 2 else p_a
                            asl2 = slice(0, NC2) if NSPLIT == 2 else csl2
                            tmp_a2 = work.tile([128, NC2], F32, tag=f"dbga{ci2}")
                            nc.vector.tensor_copy(tmp_a2[:], pa2_d[:, asl2])
                            nc.sync.dma_start(dbg["a1"][:, csl2], tmp_a2[:])
                            nc.sync.dma_start(dbg["b1"][:, csl2], st2["b1"][:])
                            nc.sync.dma_start(dbg["w"][:, csl2], st2["w"][:])
                        tmp_h2 = work.tile([128, N], F32, tag="dbgh")
                        for ci3 in range(NSPLIT):
                            nc.vector.tensor_copy(
                                tmp_h2[:, ci3 * NC2 : (ci3 + 1) * NC2],
                                p_h_ch[ci3][:, 0:NC2],
                            )
                        nc.sync.dma_start(dbg["h1"][:], tmp_h2[:])
                    gens = [
                        step_chain(n, st, db_per_chain[st["ch"]])
                        for st in chains
                    ]
                    alive = True
                    while alive:
                        alive = False
                        for g in gens:
                            try:
                                next(g)
                                alive = True
                            except StopIteration:
                                pass
                    if DEBUG and n == DEBUG_STEP:
                        for ci, st in enumerate(chains):
                            csl = slice(ci * NC2, (ci + 1) * NC2)
                            for j, ct in enumerate(st.get("cdump", [])):
                                nc.sync.dma_start(
                                    dbg[f"c{j+1}"][:, csl], ct[:]
                                )


            # ---------------- exit: RANK->DIM GEMMs, token-major output
            # per-chain copies on ACT: each chain's end-GEMMs can start as
            # soon as its own columns are final, and DVE stays free for the
            # output adds
            s_S = work.tile([128, N], F32, tag="sS")
            s_Q = work.tile([128, N], F32, tag="sQ")
            for ci in range(NSPLIT):
                csl = slice(ci * NC2, (ci + 1) * NC2)
                nc.scalar.activation(s_S[:, csl].bitcast(F32R), p_S[:, csl], COPY)
                nc.scalar.activation(s_Q[:, csl].bitcast(F32R), p_Q[:, csl], COPY)
            if DEBUG:
                nc.sync.dma_start(dbg["S"][:], s_S[:])
                nc.sync.dma_start(dbg["Q"][:], s_Q[:])

            with (
                tc.tile_pool(name="ps_end", bufs=2, space="PSUM") as ps_end,
                tc.tile_pool(name="opool", bufs=4) as opool,
            ):
                for th in range(2):  # tile half (A/B)
                    for tb4 in range(4):  # token block within half
                        tb = th * 4 + tb4
                        for dh in range(2):  # dim half
                            lhs_S = s_S[
                                th * 64 : (th + 1) * 64,
                                tb4 * 128 : (tb4 + 1) * 128,
                            ].bitcast(F32R)
                            lhs_Q = s_Q[
                                th * 64 : (th + 1) * 64,
                                tb4 * 128 : (tb4 + 1) * 128,
                            ].bitcast(F32R)
                            rv = s_wcv[
                                th * 64 : (th + 1) * 64, dh * N : (dh + 1) * N
                            ].bitcast(F32R)
                            rx = s_wcx[
                                th * 64 : (th + 1) * 64, dh * N : (dh + 1) * N
                            ].bitcast(F32R)
                            # v out
                            pv = ps_end.tile([128, N], F32, tag="eo")
                            nc.tensor.matmul(
                                pv[:],
                                lhs_S,
                                rv,
                                start=True,
                                stop=True,
                                tile_position=(64 * th, 0),
                                skip_group_check=True,
                            )
                            # v-add path off DVE: ACT copies PSUM out,
                            # GPSIMD adds (DVE keeps the x-adds)
                            pvc = opool.tile([128, N], F32, tag="pvc")
                            nc.scalar.activation(pvc[:], pv[:], COPY)
                            ov = opool.tile([128, N], F32, tag="ov")
                            nc.gpsimd.tensor_add(
                                ov[:], s_vtok[:, tb, dh * N : (dh + 1) * N].bitcast(F32), pvc[:]
                            )
                            nc.sync.dma_start(
                                vout[
                                    tb * 128 : (tb + 1) * 128,
                                    dh * N : (dh + 1) * N,
                                ],
                                ov[:],
                            )
                            # x out = (x0 + v0) + Q-gemm   (x0+v0 pre-added)
                            px = ps_end.tile([128, N], F32, tag="eo")
                            nc.tensor.matmul(
                                px[:],
                                lhs_Q,
                                rx,
                                start=True,
                                stop=True,
                                tile_position=(64 * th, 0),
                                skip_group_check=True,
                            )
                            oxh = opool.tile([128, N], F32, tag="oxh")
                            nc.vector.tensor_add(
                                oxh[:],
                                s_xtok[:, tb, dh * N : (dh + 1) * N].bitcast(F32),
                                px[:],
                            )
                            ox = opool.tile([128, N], F32, tag="ox")
                            nc.vector.tensor_add(
                                ox[:],
                                oxh[:],
                                s_vtok[:, tb, dh * N : (dh + 1) * N].bitcast(F32),
                            )
                            nc.sync.dma_start(
                                xout[
                                    tb * 128 : (tb + 1) * 128,
                                    dh * N : (dh + 1) * N,
                                ],
                                ox[:],
                            )

    orig = nc.to_json_bytes
    nc.to_json_bytes = lambda: _split_waits(orig())
    _NC_CACHE = nc
    return nc


# -------------------------------------------------------------------- driver


def _run(x, v, Wa, Wb, Wx, Wc, trace=False):
    from concourse.bass_utils import run_bass_kernel_spmd

    x = np.asarray(x, np.float32).reshape(BATCH * SEQ, DIM)
    v = np.asarray(v, np.float32).reshape(BATCH * SEQ, DIM)
    consts = _host_consts(Wa, Wb, Wx, Wc)

    nc = _build_bass()
    in_maps = []
    for c in range(NCORES):
        m = {
            "xin": np.ascontiguousarray(x[c * TPC : (c + 1) * TPC]),
            "vin": np.ascontiguousarray(v[c * TPC : (c + 1) * TPC]),
        }
        m.update(consts)
        in_maps.append(m)

    res = run_bass_kernel_spmd(
        nc, in_maps, core_ids=list(range(NCORES)), trace=trace
    )
    xo = np.concatenate([res.results[c]["xout"] for c in range(NCORES)], axis=0)
    vo = np.concatenate([res.results[c]["vout"] for c in range(NCORES)], axis=0)
    return (xo.reshape(BATCH, SEQ, DIM), vo.reshape(BATCH, SEQ, DIM)), res


def kernel(x, v, Wa, Wb, Wx, Wc):
    (xo, vo), _ = _run(x, v, Wa, Wb, Wx, Wc, trace=False)
    return xo, vo
